# revision 31
# baseline (speedup 1.0000x reference)
"""Trainium2 Bass kernel for DetectionConfidenceMap2keypoint.

Computes, for x [B=32, K=64, H=96, W=128] fp32:
  map = softmax(x, axis=1)  (channel softmax per (b, i, j))
  zeta[b,k] = sum_{i,j} map
  Sx[b,k]   = sum_{i,j} map * j
  Sy[b,k]   = sum_{i,j} map * i
  keypoint  = round(cumsum over flattened (b,k) of Sx|Sy / zeta), clamped

Strategy: pure data parallel over batch (4 batches per core, 8 cores).
Per core, batches are processed in pairs stacked on the partition dim
(2 x 64 channels = 128 partitions).  For each image row i the [128(k), 128(j)]
tile is PE-transposed to [128(j), 128(2b x k)], exp'd on ScalarE
(PSUM -> SBUF), the per-(j,b) denominators reduced on VectorE, normalized
with a stride-0-broadcast tensor_tensor multiply, PE-transposed back and
copied to an SBUF staging buffer for DMA out.  zeta/Sx/Sy come from
accumulating PE matmuls (bf16) with weights [1, j, g] against the transposed
normalized map; the tiny flattened-(b,k) cumsum epilogue runs on host.
"""

import sys

import numpy as np

for _p in ("/opt/trn_rl_repo",):
    if _p not in sys.path:
        sys.path.insert(0, _p)

B, K, H, W = 32, 64, 96, 128
NCORES = 8
BPC = B // NCORES  # batches per core
NPAIRS = BPC // 2  # batch pairs per core
RG = 8  # image rows per group (two PSUM banks)
RH = RG // 2  # rows per stats half-accumulator (one PSUM bank)
NG = H // RG  # groups per batch pair
CHUNK_ROWS = 32  # image rows per DMA chunk
GPC = CHUNK_ROWS // RG  # groups per chunk
NCHUNK = H // CHUNK_ROWS
FD = RG * W  # free dim of one group tile (1024)
FDH = RH * W  # free dim of one stats half (512)

# fp32 PE matmuls run at 1/4 rate (f32r needs producer-side rounding, which
# the BIR verifier enforces).  Instead the stats matmul runs in bf16: the
# normalized map is cast fp32->bf16 on the otherwise-idle GPSIMD engine.
# Stats are sums of ~12k bf16-rounded terms accumulated in fp32 PSUM, so the
# relative error is ~4e-3/sqrt(12288) ~ 4e-5 — verified against the
# reference in test.py (keypoints must match exactly).
STATS_BF16 = True


def _build_nc(repeat=1):
    import concourse.bass as bass
    import concourse.tile as tile
    from concourse import bacc, mybir
    from concourse.masks import make_identity

    f32 = mybir.dt.float32

    # Bacc (not raw Bass): its compile() pass splits multi-semaphore waits
    # into EventSemaphore prefix instructions — TRN2 instructions can carry
    # at most one wait, and Tile alone emits more.
    wdt = mybir.dt.bfloat16 if STATS_BF16 else f32
    nc = bacc.Bacc("TRN2", target_bir_lowering=False, debug=False)
    x = nc.dram_tensor("x", [BPC, K, H, W], f32, kind="ExternalInput").ap()
    wst = nc.dram_tensor("wst", [128, NG, 3], wdt, kind="ExternalInput").ap()
    y = nc.dram_tensor("y", [BPC, K, H, W], f32, kind="ExternalOutput").ap()
    st = nc.dram_tensor("st", [NPAIRS, 3, 2, FDH], f32, kind="ExternalOutput").ap()
    zt = nc.dram_tensor("zt", [128, NPAIRS], f32, kind="ExternalOutput").ap()

    with tile.TileContext(nc) as tc:
        for _ in range(repeat):
            _kernel_body(tc, x, wst, y, st, zt, bass, mybir, make_identity)
    nc.compile()
    return nc


def _kernel_body(tc, x, wst, y, st, zt, bass, mybir, make_identity):
    nc = tc.nc
    f32 = mybir.dt.float32
    bf16 = mybir.dt.bfloat16
    Exp = mybir.ActivationFunctionType.Exp
    Copy = mybir.ActivationFunctionType.Copy
    X = mybir.AxisListType.X
    mult = mybir.AluOpType.mult

    import contextlib

    ctx = contextlib.ExitStack()
    with ctx:
        consts = ctx.enter_context(tc.tile_pool(name="consts", bufs=1))
        inpool = ctx.enter_context(tc.tile_pool(name="inpool", bufs=3))
        epool = ctx.enter_context(tc.tile_pool(name="epool", bufs=4))
        mpool = ctx.enter_context(tc.tile_pool(name="mpool", bufs=4))
        dpool = ctx.enter_context(tc.tile_pool(name="dpool", bufs=8))
        bfpool = ctx.enter_context(tc.tile_pool(name="bfpool", bufs=4))
        stagepool = ctx.enter_context(tc.tile_pool(name="stagepool", bufs=3))
        statsb = ctx.enter_context(tc.tile_pool(name="statsb", bufs=1))
        # PSUM bank budget (8 banks): psumf 2x2 + psumb 2x1 + stats 2x1.
        psumf = ctx.enter_context(tc.tile_pool(name="psumf", bufs=2, space="PSUM"))
        psumb = ctx.enter_context(tc.tile_pool(name="psumb", bufs=2, space="PSUM"))
        psums = ctx.enter_context(tc.tile_pool(name="psums", bufs=2, space="PSUM"))

        wdt = bf16 if STATS_BF16 else f32
        ident = consts.tile([128, 128], f32)
        make_identity(nc, ident)
        wst_sb = consts.tile([128, NG, 3], wdt)
        nc.sync.dma_start(out=wst_sb, in_=wst)
        stats_sb = statsb.tile([3, NPAIRS, 2, FDH], f32)
        zeta_sb = statsb.tile([128, NPAIRS], f32)

        pending = None  # phase-B work item carried across one group

        def phase_b(it):
            c = it["c"]
            for h in range(2):
                # stats matmul first: it carries the wait on the stats rhs, so
                # the back transposes below need no extra foreign-sem waits.
                nc.tensor.matmul(
                    it["stats_ps"][h],
                    it["w"],
                    it["stats_rhs"][:, h * RH : (h + 1) * RH, :].rearrange(
                        "p a b -> p (a b)"
                    ),
                    start=(it["g"] == 0),
                    stop=(it["g"] == NG - 1),
                )
                psb = psumb.tile([128, RH, 128], f32, tag="psb")
                for r in range(RH):
                    nc.tensor.transpose(
                        psb[:, r, :], it["mapT"][:, h * RH + r, :], ident
                    )
                # the stage copy doubles as the fp32 zeta accumulator: the
                # per-partition (= per (b,k)) sum of the copied map lands in
                # one column of zparts for free.
                nc.scalar.activation(
                    out=it["stage"][
                        :, it["gg"] * RG + h * RH : it["gg"] * RG + (h + 1) * RH, :
                    ],
                    in_=psb,
                    func=Copy,
                    accum_out=it["zp"][:, 2 * it["g"] + h : 2 * it["g"] + h + 1],
                )
            if it["g"] == NG - 1:
                for h in range(2):
                    nc.scalar.copy(out=stats_sb[:, c, h, :], in_=it["stats_ps"][h])
                nc.sync.dma_start(out=st[c], in_=stats_sb[:, c, :, :])
                nc.vector.reduce_sum(
                    out=zeta_sb[:, c : c + 1], in_=it["zp"], axis=X
                )
                if c == NPAIRS - 1:
                    nc.sync.dma_start(out=zt, in_=zeta_sb)
            if it["gg"] == GPC - 1:
                ch = it["ch"]
                nc.sync.dma_start(
                    out=y[
                        2 * c : 2 * c + 2, :, ch * CHUNK_ROWS : (ch + 1) * CHUNK_ROWS, :
                    ].rearrange("b k r w -> (b k) r w"),
                    in_=it["stage"],
                )

        for c in range(NPAIRS):
            stats_ps = [
                psums.tile([3, FDH], f32, tag="stats", name=f"stats_ps{c}_{h}")
                for h in range(2)
            ]
            zp = dpool.tile([128, 2 * NG], f32, tag="zp")
            for ch in range(NCHUNK):
                in_t = inpool.tile([128, CHUNK_ROWS, W], f32)
                nc.sync.dma_start(
                    out=in_t,
                    in_=x[
                        2 * c : 2 * c + 2, :, ch * CHUNK_ROWS : (ch + 1) * CHUNK_ROWS, :
                    ].rearrange("b k r w -> (b k) r w"),
                )
                stage = stagepool.tile([128, CHUNK_ROWS, W], f32)
                for gg in range(GPC):
                    g = ch * GPC + gg
                    # ---- phase A: fwd transpose, exp, denom, recip, normalize
                    psf = psumf.tile([128, RG, 128], f32)
                    for r in range(RG):
                        nc.tensor.transpose(
                            psf[:, r, :], in_t[:, gg * RG + r, :], ident
                        )
                    eT = epool.tile([128, RG, 128], f32)
                    nc.scalar.activation(out=eT, in_=psf, func=Exp)
                    den = dpool.tile([128, 2 * RG], f32)
                    nc.vector.reduce_sum(
                        out=den,
                        in_=eT.rearrange("p a (s k) -> p (a s) k", k=64),
                        axis=X,
                    )
                    rcp = dpool.tile([128, 2 * RG], f32)
                    nc.vector.reciprocal(out=rcp, in_=den)
                    mapT = mpool.tile([128, RG, 128], f32)
                    nc.vector.tensor_tensor(
                        out=mapT.rearrange("p a b -> p (a b)"),
                        in0=eT.rearrange("p a b -> p (a b)"),
                        in1=rcp.unsqueeze(2).to_broadcast([128, 2 * RG, 64]),
                        op=mult,
                    )
                    if STATS_BF16:
                        stats_rhs = bfpool.tile([128, RG, 128], bf16)
                        nc.gpsimd.tensor_copy(out=stats_rhs, in_=mapT)
                    else:
                        stats_rhs = mapT
                    item = dict(
                        mapT=mapT,
                        stats_rhs=stats_rhs,
                        stage=stage,
                        stats_ps=stats_ps,
                        zp=zp,
                        w=wst_sb[:, g, :],
                        g=g,
                        gg=gg,
                        ch=ch,
                        c=c,
                    )
                    # ---- phase B of the previous group (software pipeline)
                    if pending is not None:
                        phase_b(pending)
                    pending = item
        phase_b(pending)


_CACHE = {}


def _get_nc():
    if "nc" not in _CACHE:
        _CACHE["nc"] = _build_nc()
    return _CACHE["nc"]


def _host_weights():
    w = np.zeros((128, NG, 3), dtype=np.float32)
    w[:, :, 0] = 1.0
    w[:, :, 1] = np.arange(128, dtype=np.float32)[:, None]
    w[:, :, 2] = np.arange(NG, dtype=np.float32)[None, :]
    if STATS_BF16:
        import ml_dtypes

        w = w.astype(ml_dtypes.bfloat16)  # 1, j<=127, g<=11: all exact in bf16
    return w


def run_device(x, trace=False, tmpdir=None):
    """Run the bass kernel on 8 cores. x: [B,K,H,W] fp32.

    Returns (map_out [B,K,H,W], zeta, Sx, Sy  each [B,K]), exec_time_ns."""
    from concourse.bass_utils import run_bass_kernel_spmd

    nc = _get_nc()
    wst = _host_weights()
    in_maps = [
        {"x": np.ascontiguousarray(x[BPC * m : BPC * (m + 1)]), "wst": wst}
        for m in range(NCORES)
    ]
    kw = {}
    if tmpdir is not None:
        kw["tmpdir"] = tmpdir
    res = run_bass_kernel_spmd(
        nc, in_maps, core_ids=list(range(NCORES)), trace=trace, **kw
    )
    map_out = np.concatenate([r["y"] for r in res.results], axis=0)
    zeta, Sx, Sy = decode_stats([r["st"] for r in res.results])
    # overwrite zeta with the exact-fp32 accumulator path
    for m, r in enumerate(res.results):
        ztm = r["zt"].reshape(2, K, NPAIRS)
        for c in range(NPAIRS):
            for hb in range(2):
                zeta[BPC * m + 2 * c + hb] = ztm[hb, :, c]
    return map_out, zeta, Sx, Sy, res.exec_time_ns


def decode_stats(st_list):
    """st per core: [NPAIRS, 3, 2(half), FDH] where the FDH cols are
    (r' in RH, b in 2, k in 64) and the accumulators are summed over groups g
    with weights [1, j, g].  Row index i = RG*g + RH*h + r'."""
    zeta = np.empty((B, K), np.float32)
    Sx = np.empty((B, K), np.float32)
    Sy = np.empty((B, K), np.float32)
    roff = (
        np.float32(RH) * np.arange(2, dtype=np.float32)[:, None]
        + np.arange(RH, dtype=np.float32)[None, :]
    )
    for m, st in enumerate(st_list):
        stm = st.reshape(NPAIRS, 3, 2, RH, 2, K)
        for c in range(NPAIRS):
            for hb in range(2):
                b = BPC * m + 2 * c + hb
                S0 = stm[c, 0, :, :, hb, :]  # [2, RH, K]
                S1 = stm[c, 1, :, :, hb, :]
                S2 = stm[c, 2, :, :, hb, :]
                zeta[b] = S0.sum(axis=(0, 1))
                Sx[b] = S1.sum(axis=(0, 1))
                Sy[b] = (np.float32(RG) * S2 + roff[:, :, None] * S0).sum(axis=(0, 1))
    return zeta, Sx, Sy


def _epilogue(zeta, Sx, Sy):
    cum_x = np.cumsum(Sx.reshape(-1), dtype=np.float32).reshape(B, K)
    cum_y = np.cumsum(Sy.reshape(-1), dtype=np.float32).reshape(B, K)
    kx = np.round(cum_x / zeta)
    ky = np.round(cum_y / zeta)
    kx = np.where((kx > W) | (kx < 0), np.float32(W * 0.5), kx).astype(np.float32)
    ky = np.where((ky > H) | (ky < 0), np.float32(H * 0.5), ky).astype(np.float32)
    return np.stack([kx, ky], axis=-1).astype(np.float32)


def kernel(combined_hm_preds, batch_size=B, num_of_kp=K, **_unused):
    x = np.ascontiguousarray(np.asarray(combined_hm_preds), dtype=np.float32)
    assert x.shape == (B, K, H, W), x.shape
    map_out, zeta, Sx, Sy, _ = run_device(x)
    keypoint = _epilogue(zeta, Sx, Sy)
    return map_out, keypoint, zeta


# revision 35
# speedup vs baseline: 1.1803x; 1.1803x over previous
"""Trainium2 Bass kernel for DetectionConfidenceMap2keypoint.

Computes, for x [B=32, K=64, H=96, W=128] fp32:
  map = softmax(x, axis=1)  (channel softmax per (b, i, j))
  zeta[b,k] = sum_{i,j} map
  Sx[b,k]   = sum_{i,j} map * j
  Sy[b,k]   = sum_{i,j} map * i
  keypoint  = round(cumsum over flattened (b,k) of Sx|Sy / zeta), clamped

Strategy: pure data parallel over batch (4 batches per core, 8 cores).
Per core, batches are processed in pairs stacked on the partition dim
(2 x 64 channels = 128 partitions).  For each image row i the [128(k), 128(j)]
tile is PE-transposed to [128(j), 128(2b x k)], exp'd on ScalarE
(PSUM -> SBUF), the per-(j,b) denominators reduced on VectorE, normalized
with a stride-0-broadcast tensor_tensor multiply, PE-transposed back and
copied to an SBUF staging buffer for DMA out.  zeta/Sx/Sy come from
accumulating PE matmuls (bf16) with weights [1, j, g] against the transposed
normalized map; the tiny flattened-(b,k) cumsum epilogue runs on host.
"""

import sys

import numpy as np

for _p in ("/opt/trn_rl_repo",):
    if _p not in sys.path:
        sys.path.insert(0, _p)

B, K, H, W = 32, 64, 96, 128
NCORES = 8
BPC = B // NCORES  # batches per core
NPAIRS = BPC // 2  # batch pairs per core
RG = 8  # image rows per group (two PSUM banks)
RH = RG // 2  # rows per stats half-accumulator (one PSUM bank)
NG = H // RG  # groups per batch pair
CHUNK_ROWS = 16  # image rows per DMA chunk
GPC = CHUNK_ROWS // RG  # groups per chunk
NCHUNK = H // CHUNK_ROWS
FD = RG * W  # free dim of one group tile (1024)
FDH = RH * W  # free dim of one stats half (512)

# fp32 PE matmuls run at 1/4 rate (f32r needs producer-side rounding, which
# the BIR verifier enforces).  Instead the stats matmul runs in bf16: the
# normalized map is cast fp32->bf16 on the otherwise-idle GPSIMD engine.
# Stats are sums of ~12k bf16-rounded terms accumulated in fp32 PSUM, so the
# relative error is ~4e-3/sqrt(12288) ~ 4e-5 — verified against the
# reference in test.py (keypoints must match exactly).
STATS_BF16 = True


def _build_nc(repeat=1):
    import concourse.bass as bass
    import concourse.tile as tile
    from concourse import bacc, mybir
    from concourse.masks import make_identity

    f32 = mybir.dt.float32

    # Bacc (not raw Bass): its compile() pass splits multi-semaphore waits
    # into EventSemaphore prefix instructions — TRN2 instructions can carry
    # at most one wait, and Tile alone emits more.
    wdt = mybir.dt.bfloat16 if STATS_BF16 else f32
    nc = bacc.Bacc("TRN2", target_bir_lowering=False, debug=False)
    x = nc.dram_tensor("x", [BPC, K, H, W], f32, kind="ExternalInput").ap()
    wst = nc.dram_tensor("wst", [128, NG, 3], wdt, kind="ExternalInput").ap()
    y = nc.dram_tensor("y", [BPC, K, H, W], f32, kind="ExternalOutput").ap()
    st = nc.dram_tensor("st", [NPAIRS, 3, 2, FDH], f32, kind="ExternalOutput").ap()
    zt = nc.dram_tensor("zt", [128, NPAIRS], f32, kind="ExternalOutput").ap()

    with tile.TileContext(nc) as tc:
        for _ in range(repeat):
            _kernel_body(tc, x, wst, y, st, zt, bass, mybir, make_identity)
    nc.compile()
    return nc


def _kernel_body(tc, x, wst, y, st, zt, bass, mybir, make_identity):
    nc = tc.nc
    f32 = mybir.dt.float32
    bf16 = mybir.dt.bfloat16
    Exp = mybir.ActivationFunctionType.Exp
    Copy = mybir.ActivationFunctionType.Copy
    X = mybir.AxisListType.X
    mult = mybir.AluOpType.mult

    import contextlib

    ctx = contextlib.ExitStack()
    with ctx:
        consts = ctx.enter_context(tc.tile_pool(name="consts", bufs=1))
        inpool = ctx.enter_context(tc.tile_pool(name="inpool", bufs=4))
        epool = ctx.enter_context(tc.tile_pool(name="epool", bufs=6))
        mpool = ctx.enter_context(tc.tile_pool(name="mpool", bufs=6))
        dpool = ctx.enter_context(tc.tile_pool(name="dpool", bufs=8))
        bfpool = ctx.enter_context(tc.tile_pool(name="bfpool", bufs=6))
        stagepool = ctx.enter_context(tc.tile_pool(name="stagepool", bufs=4))
        statsb = ctx.enter_context(tc.tile_pool(name="statsb", bufs=1))
        # PSUM bank budget (8 banks): psumf 2x2 + psumb 2x1 + stats 2x1.
        psumf = ctx.enter_context(tc.tile_pool(name="psumf", bufs=2, space="PSUM"))
        psumb = ctx.enter_context(tc.tile_pool(name="psumb", bufs=2, space="PSUM"))
        psums = ctx.enter_context(tc.tile_pool(name="psums", bufs=2, space="PSUM"))

        wdt = bf16 if STATS_BF16 else f32
        ident = consts.tile([128, 128], f32)
        make_identity(nc, ident)
        wst_sb = consts.tile([128, NG, 3], wdt)
        nc.sync.dma_start(out=wst_sb, in_=wst)
        stats_sb = statsb.tile([3, NPAIRS, 2, FDH], f32)
        zeta_sb = statsb.tile([128, NPAIRS], f32)

        pending = None  # phase-B work item carried across one group

        def phase_b(it):
            c = it["c"]
            for h in range(2):
                # stats matmul first: it carries the wait on the stats rhs, so
                # the back transposes below need no extra foreign-sem waits.
                nc.tensor.matmul(
                    it["stats_ps"][h],
                    it["w"],
                    it["stats_rhs"][:, h * RH : (h + 1) * RH, :].rearrange(
                        "p a b -> p (a b)"
                    ),
                    start=(it["g"] == 0),
                    stop=(it["g"] == NG - 1),
                )
                psb = psumb.tile([128, RH, 128], f32, tag="psb")
                for r in range(RH):
                    nc.tensor.transpose(
                        psb[:, r, :], it["mapT"][:, h * RH + r, :], ident
                    )
                # the stage copy doubles as the fp32 zeta accumulator: the
                # per-partition (= per (b,k)) sum of the copied map lands in
                # one column of zparts for free.
                nc.scalar.activation(
                    out=it["stage"][
                        :, it["gg"] * RG + h * RH : it["gg"] * RG + (h + 1) * RH, :
                    ],
                    in_=psb,
                    func=Copy,
                    accum_out=it["zp"][:, 2 * it["g"] + h : 2 * it["g"] + h + 1],
                )
            if it["g"] == NG - 1:
                for h in range(2):
                    nc.scalar.copy(out=stats_sb[:, c, h, :], in_=it["stats_ps"][h])
                nc.sync.dma_start(out=st[c], in_=stats_sb[:, c, :, :])
                nc.vector.reduce_sum(
                    out=zeta_sb[:, c : c + 1], in_=it["zp"], axis=X
                )
                if c == NPAIRS - 1:
                    nc.sync.dma_start(out=zt, in_=zeta_sb)
            if it["gg"] == GPC - 1:
                ch = it["ch"]
                nc.sync.dma_start(
                    out=y[
                        2 * c : 2 * c + 2, :, ch * CHUNK_ROWS : (ch + 1) * CHUNK_ROWS, :
                    ].rearrange("b k r w -> (b k) r w"),
                    in_=it["stage"],
                )

        for c in range(NPAIRS):
            stats_ps = [
                psums.tile([3, FDH], f32, tag="stats", name=f"stats_ps{c}_{h}")
                for h in range(2)
            ]
            zp = dpool.tile([128, 2 * NG], f32, tag="zp")
            for ch in range(NCHUNK):
                in_t = inpool.tile([128, CHUNK_ROWS, W], f32)
                nc.sync.dma_start(
                    out=in_t,
                    in_=x[
                        2 * c : 2 * c + 2, :, ch * CHUNK_ROWS : (ch + 1) * CHUNK_ROWS, :
                    ].rearrange("b k r w -> (b k) r w"),
                )
                stage = stagepool.tile([128, CHUNK_ROWS, W], f32)
                for gg in range(GPC):
                    g = ch * GPC + gg
                    # ---- phase A: fwd transpose, exp, denom, recip, normalize
                    psf = psumf.tile([128, RG, 128], f32)
                    for r in range(RG):
                        nc.tensor.transpose(
                            psf[:, r, :], in_t[:, gg * RG + r, :], ident
                        )
                    eT = epool.tile([128, RG, 128], f32)
                    nc.scalar.activation(out=eT, in_=psf, func=Exp)
                    den = dpool.tile([128, 2 * RG], f32)
                    nc.vector.reduce_sum(
                        out=den,
                        in_=eT.rearrange("p a (s k) -> p (a s) k", k=64),
                        axis=X,
                    )
                    rcp = dpool.tile([128, 2 * RG], f32)
                    nc.vector.reciprocal(out=rcp, in_=den)
                    mapT = mpool.tile([128, RG, 128], f32)
                    nc.vector.tensor_tensor(
                        out=mapT.rearrange("p a b -> p (a b)"),
                        in0=eT.rearrange("p a b -> p (a b)"),
                        in1=rcp.unsqueeze(2).to_broadcast([128, 2 * RG, 64]),
                        op=mult,
                    )
                    if STATS_BF16:
                        stats_rhs = bfpool.tile([128, RG, 128], bf16)
                        nc.gpsimd.tensor_copy(out=stats_rhs, in_=mapT)
                    else:
                        stats_rhs = mapT
                    item = dict(
                        mapT=mapT,
                        stats_rhs=stats_rhs,
                        stage=stage,
                        stats_ps=stats_ps,
                        zp=zp,
                        w=wst_sb[:, g, :],
                        g=g,
                        gg=gg,
                        ch=ch,
                        c=c,
                    )
                    # ---- phase B of the previous group (software pipeline)
                    if pending is not None:
                        phase_b(pending)
                    pending = item
        phase_b(pending)


_CACHE = {}


def _get_nc():
    if "nc" not in _CACHE:
        _CACHE["nc"] = _build_nc()
    return _CACHE["nc"]


def _host_weights():
    w = np.zeros((128, NG, 3), dtype=np.float32)
    w[:, :, 0] = 1.0
    w[:, :, 1] = np.arange(128, dtype=np.float32)[:, None]
    w[:, :, 2] = np.arange(NG, dtype=np.float32)[None, :]
    if STATS_BF16:
        import ml_dtypes

        w = w.astype(ml_dtypes.bfloat16)  # 1, j<=127, g<=11: all exact in bf16
    return w


def run_device(x, trace=False, tmpdir=None):
    """Run the bass kernel on 8 cores. x: [B,K,H,W] fp32.

    Returns (map_out [B,K,H,W], zeta, Sx, Sy  each [B,K]), exec_time_ns."""
    from concourse.bass_utils import run_bass_kernel_spmd

    nc = _get_nc()
    wst = _host_weights()
    in_maps = [
        {"x": np.ascontiguousarray(x[BPC * m : BPC * (m + 1)]), "wst": wst}
        for m in range(NCORES)
    ]
    kw = {}
    if tmpdir is not None:
        kw["tmpdir"] = tmpdir
    res = run_bass_kernel_spmd(
        nc, in_maps, core_ids=list(range(NCORES)), trace=trace, **kw
    )
    map_out = np.concatenate([r["y"] for r in res.results], axis=0)
    zeta, Sx, Sy = decode_stats([r["st"] for r in res.results])
    # overwrite zeta with the exact-fp32 accumulator path
    for m, r in enumerate(res.results):
        ztm = r["zt"].reshape(2, K, NPAIRS)
        for c in range(NPAIRS):
            for hb in range(2):
                zeta[BPC * m + 2 * c + hb] = ztm[hb, :, c]
    return map_out, zeta, Sx, Sy, res.exec_time_ns


def decode_stats(st_list):
    """st per core: [NPAIRS, 3, 2(half), FDH] where the FDH cols are
    (r' in RH, b in 2, k in 64) and the accumulators are summed over groups g
    with weights [1, j, g].  Row index i = RG*g + RH*h + r'."""
    zeta = np.empty((B, K), np.float32)
    Sx = np.empty((B, K), np.float32)
    Sy = np.empty((B, K), np.float32)
    roff = (
        np.float32(RH) * np.arange(2, dtype=np.float32)[:, None]
        + np.arange(RH, dtype=np.float32)[None, :]
    )
    for m, st in enumerate(st_list):
        stm = st.reshape(NPAIRS, 3, 2, RH, 2, K)
        for c in range(NPAIRS):
            for hb in range(2):
                b = BPC * m + 2 * c + hb
                S0 = stm[c, 0, :, :, hb, :]  # [2, RH, K]
                S1 = stm[c, 1, :, :, hb, :]
                S2 = stm[c, 2, :, :, hb, :]
                zeta[b] = S0.sum(axis=(0, 1))
                Sx[b] = S1.sum(axis=(0, 1))
                Sy[b] = (np.float32(RG) * S2 + roff[:, :, None] * S0).sum(axis=(0, 1))
    return zeta, Sx, Sy


def _epilogue(zeta, Sx, Sy):
    cum_x = np.cumsum(Sx.reshape(-1), dtype=np.float32).reshape(B, K)
    cum_y = np.cumsum(Sy.reshape(-1), dtype=np.float32).reshape(B, K)
    kx = np.round(cum_x / zeta)
    ky = np.round(cum_y / zeta)
    kx = np.where((kx > W) | (kx < 0), np.float32(W * 0.5), kx).astype(np.float32)
    ky = np.where((ky > H) | (ky < 0), np.float32(H * 0.5), ky).astype(np.float32)
    return np.stack([kx, ky], axis=-1).astype(np.float32)


def kernel(combined_hm_preds, batch_size=B, num_of_kp=K, **_unused):
    x = np.ascontiguousarray(np.asarray(combined_hm_preds), dtype=np.float32)
    assert x.shape == (B, K, H, W), x.shape
    map_out, zeta, Sx, Sy, _ = run_device(x)
    keypoint = _epilogue(zeta, Sx, Sy)
    return map_out, keypoint, zeta


# revision 40
# speedup vs baseline: 1.8266x; 1.5476x over previous
"""Trainium2 Bass kernel for DetectionConfidenceMap2keypoint.

Computes, for x [B=32, K=64, H=96, W=128] fp32:
  map = softmax(x, axis=1)  (channel softmax per (b, i, j))
  zeta[b,k] = sum_{i,j} map
  Sx[b,k]   = sum_{i,j} map * j
  Sy[b,k]   = sum_{i,j} map * i
  keypoint  = round(cumsum over flattened (b,k) of Sx|Sy / zeta), clamped

Strategy: pure data parallel over batch (4 batches per core, 8 cores).
Per core, batches are processed in pairs stacked on the partition dim
(2 x 64 channels = 128 partitions).  For each image row i the [128(k), 128(j)]
tile is PE-transposed to [128(j), 128(2b x k)], exp'd on ScalarE
(PSUM -> SBUF), the per-(j,b) denominators reduced on VectorE, normalized
with a stride-0-broadcast tensor_tensor multiply, PE-transposed back and
copied to an SBUF staging buffer for DMA out.  zeta/Sx/Sy come from
accumulating PE matmuls (bf16) with weights [1, j, g] against the transposed
normalized map; the tiny flattened-(b,k) cumsum epilogue runs on host.
"""

import sys

import numpy as np

for _p in ("/opt/trn_rl_repo",):
    if _p not in sys.path:
        sys.path.insert(0, _p)

B, K, H, W = 32, 64, 96, 128
NCORES = 8
BPC = B // NCORES  # batches per core
NPAIRS = BPC // 2  # batch pairs per core
RG = 8  # image rows per group (two PSUM banks)
RH = RG // 2  # rows per stats half-accumulator (one PSUM bank)
NG = H // RG  # groups per batch pair
CHUNK_ROWS = 16  # image rows per DMA chunk
GPC = CHUNK_ROWS // RG  # groups per chunk
NCHUNK = H // CHUNK_ROWS
FD = RG * W  # free dim of one group tile (1024)
FDH = RH * W  # free dim of one stats half (512)

# fp32 PE matmuls run at 1/4 rate (f32r needs producer-side rounding, which
# the BIR verifier enforces).  Instead the stats matmul runs in bf16: the
# normalized map is cast fp32->bf16 on the otherwise-idle GPSIMD engine.
# Stats are sums of ~12k bf16-rounded terms accumulated in fp32 PSUM, so the
# relative error is ~4e-3/sqrt(12288) ~ 4e-5 — verified against the
# reference in test.py (keypoints must match exactly).
STATS_BF16 = True


def _build_nc(repeat=1):
    import concourse.bass as bass
    import concourse.tile as tile
    from concourse import bacc, mybir
    from concourse.masks import make_identity

    f32 = mybir.dt.float32

    # Bacc (not raw Bass): its compile() pass splits multi-semaphore waits
    # into EventSemaphore prefix instructions — TRN2 instructions can carry
    # at most one wait, and Tile alone emits more.
    wdt = mybir.dt.bfloat16 if STATS_BF16 else f32
    nc = bacc.Bacc("TRN2", target_bir_lowering=False, debug=False)
    x = nc.dram_tensor("x", [BPC, K, H, W], f32, kind="ExternalInput").ap()
    wst = nc.dram_tensor("wst", [128, NG, 3], wdt, kind="ExternalInput").ap()
    y = nc.dram_tensor("y", [BPC, K, H, W], f32, kind="ExternalOutput").ap()
    st = nc.dram_tensor("st", [NPAIRS, 3, 2, FDH], f32, kind="ExternalOutput").ap()
    zt = nc.dram_tensor("zt", [128, NPAIRS], f32, kind="ExternalOutput").ap()

    with tile.TileContext(nc) as tc:
        for _ in range(repeat):
            _kernel_body(tc, x, wst, y, st, zt, bass, mybir, make_identity)
    nc.compile()
    return nc


def _kernel_body(tc, x, wst, y, st, zt, bass, mybir, make_identity):
    nc = tc.nc
    f32 = mybir.dt.float32
    bf16 = mybir.dt.bfloat16
    Exp = mybir.ActivationFunctionType.Exp
    Copy = mybir.ActivationFunctionType.Copy
    X = mybir.AxisListType.X
    mult = mybir.AluOpType.mult

    import contextlib

    ctx = contextlib.ExitStack()
    with ctx:
        consts = ctx.enter_context(tc.tile_pool(name="consts", bufs=1))
        inpool = ctx.enter_context(tc.tile_pool(name="inpool", bufs=4))
        epool = ctx.enter_context(tc.tile_pool(name="epool", bufs=6))
        mpool = ctx.enter_context(tc.tile_pool(name="mpool", bufs=6))
        dpool = ctx.enter_context(tc.tile_pool(name="dpool", bufs=8))
        bfpool = ctx.enter_context(tc.tile_pool(name="bfpool", bufs=6))
        stagepool = ctx.enter_context(tc.tile_pool(name="stagepool", bufs=4))
        statsb = ctx.enter_context(tc.tile_pool(name="statsb", bufs=1))
        # PSUM bank budget (8 banks): psumf 2x2 + psumb 2x1 + stats 2x1.
        psumf = ctx.enter_context(tc.tile_pool(name="psumf", bufs=2, space="PSUM"))
        psumb = ctx.enter_context(tc.tile_pool(name="psumb", bufs=2, space="PSUM"))
        psums = ctx.enter_context(tc.tile_pool(name="psums", bufs=2, space="PSUM"))

        wdt = bf16 if STATS_BF16 else f32
        ident = consts.tile([128, 128], f32)
        make_identity(nc, ident)
        wst_sb = consts.tile([128, NG, 3], wdt)
        nc.sync.dma_start(out=wst_sb, in_=wst)
        stats_sb = statsb.tile([3, NPAIRS, 2, FDH], f32)
        zeta_sb = statsb.tile([128, NPAIRS], f32)

        pending = None  # phase-B work item carried across one group

        def phase_b(it):
            c = it["c"]
            for h in range(2):
                # stats matmul first: it carries the wait on the stats rhs, so
                # the back transposes below need no extra foreign-sem waits.
                nc.tensor.matmul(
                    it["stats_ps"][h],
                    it["w"],
                    it["stats_rhs"][:, h * RH : (h + 1) * RH, :].rearrange(
                        "p a b -> p (a b)"
                    ),
                    start=(it["g"] == 0),
                    stop=(it["g"] == NG - 1),
                )
                psb = psumb.tile([128, RH, 128], f32, tag="psb")
                for r in range(RH):
                    nc.tensor.transpose(
                        psb[:, r, :], it["mapT"][:, h * RH + r, :], ident
                    )
                # the stage copy doubles as the fp32 zeta accumulator: the
                # per-partition (= per (b,k)) sum of the copied map lands in
                # one column of zparts for free.
                nc.scalar.activation(
                    out=it["stage"][
                        :, it["gg"] * RG + h * RH : it["gg"] * RG + (h + 1) * RH, :
                    ],
                    in_=psb,
                    func=Copy,
                    accum_out=it["zp"][:, 2 * it["g"] + h : 2 * it["g"] + h + 1],
                )
            if it["g"] == NG - 1:
                for h in range(2):
                    nc.scalar.copy(out=stats_sb[:, c, h, :], in_=it["stats_ps"][h])
                nc.sync.dma_start(out=st[c], in_=stats_sb[:, c, :, :])
                nc.vector.reduce_sum(
                    out=zeta_sb[:, c : c + 1], in_=it["zp"], axis=X
                )
                if c == NPAIRS - 1:
                    nc.sync.dma_start(out=zt, in_=zeta_sb)
            if it["gg"] == GPC - 1:
                ch = it["ch"]
                nc.sync.dma_start(
                    out=y[
                        2 * c : 2 * c + 2, :, ch * CHUNK_ROWS : (ch + 1) * CHUNK_ROWS, :
                    ].rearrange("b k r w -> (b k) r w"),
                    in_=it["stage"],
                )

        for c in range(NPAIRS):
            stats_ps = [
                psums.tile([3, FDH], f32, tag="stats", name=f"stats_ps{c}_{h}")
                for h in range(2)
            ]
            zp = dpool.tile([128, 2 * NG], f32, tag="zp")
            for ch in range(NCHUNK):
                in_t = inpool.tile([128, CHUNK_ROWS, W], f32)
                nc.sync.dma_start(
                    out=in_t,
                    in_=x[
                        2 * c : 2 * c + 2, :, ch * CHUNK_ROWS : (ch + 1) * CHUNK_ROWS, :
                    ].rearrange("b k r w -> (b k) r w"),
                )
                stage = stagepool.tile([128, CHUNK_ROWS, W], f32)
                for gg in range(GPC):
                    g = ch * GPC + gg
                    # ---- phase A: fwd transpose, exp, denom, recip, normalize
                    psf = psumf.tile([128, RG, 128], f32)
                    for r in range(RG):
                        nc.tensor.transpose(
                            psf[:, r, :], in_t[:, gg * RG + r, :], ident
                        )
                    eT = epool.tile([128, RG, 128], f32)
                    nc.scalar.activation(out=eT, in_=psf, func=Exp)
                    den = dpool.tile([128, 2 * RG], f32)
                    nc.vector.reduce_sum(
                        out=den,
                        in_=eT.rearrange("p a (s k) -> p (a s) k", k=64),
                        axis=X,
                    )
                    rcp = dpool.tile([128, 2 * RG], f32)
                    nc.vector.reciprocal(out=rcp, in_=den)
                    mapT = mpool.tile([128, RG, 128], f32)
                    nc.vector.tensor_tensor(
                        out=mapT.rearrange("p a b -> p (a b)"),
                        in0=eT.rearrange("p a b -> p (a b)"),
                        in1=rcp.unsqueeze(2).to_broadcast([128, 2 * RG, 64]),
                        op=mult,
                    )
                    if STATS_BF16:
                        stats_rhs = bfpool.tile([128, RG, 128], bf16)
                        nc.gpsimd.tensor_copy(out=stats_rhs, in_=mapT)
                    else:
                        stats_rhs = mapT
                    item = dict(
                        mapT=mapT,
                        stats_rhs=stats_rhs,
                        stage=stage,
                        stats_ps=stats_ps,
                        zp=zp,
                        w=wst_sb[:, g, :],
                        g=g,
                        gg=gg,
                        ch=ch,
                        c=c,
                    )
                    # ---- phase B of the previous group (software pipeline)
                    if pending is not None:
                        phase_b(pending)
                    pending = item
        phase_b(pending)


_CACHE = {}


def _get_nc():
    if "nc" not in _CACHE:
        _CACHE["nc"] = _build_nc()
    return _CACHE["nc"]


def _host_weights():
    w = np.zeros((128, NG, 3), dtype=np.float32)
    w[:, :, 0] = 1.0
    w[:, :, 1] = np.arange(128, dtype=np.float32)[:, None]
    w[:, :, 2] = np.arange(NG, dtype=np.float32)[None, :]
    if STATS_BF16:
        import ml_dtypes

        w = w.astype(ml_dtypes.bfloat16)  # 1, j<=127, g<=11: all exact in bf16
    return w


def run_device(x, trace=False, tmpdir=None):
    """Run the bass kernel on 8 cores. x: [B,K,H,W] fp32.

    Returns (map_out [B,K,H,W], zeta, Sx, Sy  each [B,K]), exec_time_ns."""
    from concourse.bass_utils import run_bass_kernel_spmd

    nc = _get_nc()
    wst = _host_weights()
    in_maps = [
        {"x": np.ascontiguousarray(x[BPC * m : BPC * (m + 1)]), "wst": wst}
        for m in range(NCORES)
    ]
    kw = {}
    if tmpdir is not None:
        kw["tmpdir"] = tmpdir
    res = run_bass_kernel_spmd(
        nc, in_maps, core_ids=list(range(NCORES)), trace=trace, **kw
    )
    map_out = np.concatenate([r["y"] for r in res.results], axis=0)
    zeta, Sx, Sy = decode_stats([r["st"] for r in res.results])
    # overwrite zeta with the exact-fp32 accumulator path
    for m, r in enumerate(res.results):
        ztm = r["zt"].reshape(2, K, NPAIRS)
        for c in range(NPAIRS):
            for hb in range(2):
                zeta[BPC * m + 2 * c + hb] = ztm[hb, :, c]
    return map_out, zeta, Sx, Sy, res.exec_time_ns


def decode_stats(st_list):
    """st per core: [NPAIRS, 3, 2(half), FDH] where the FDH cols are
    (r' in RH, b in 2, k in 64) and the accumulators are summed over groups g
    with weights [1, j, g].  Row index i = RG*g + RH*h + r'."""
    zeta = np.empty((B, K), np.float32)
    Sx = np.empty((B, K), np.float32)
    Sy = np.empty((B, K), np.float32)
    roff = (
        np.float32(RH) * np.arange(2, dtype=np.float32)[:, None]
        + np.arange(RH, dtype=np.float32)[None, :]
    )
    for m, st in enumerate(st_list):
        stm = st.reshape(NPAIRS, 3, 2, RH, 2, K)
        for c in range(NPAIRS):
            for hb in range(2):
                b = BPC * m + 2 * c + hb
                S0 = stm[c, 0, :, :, hb, :]  # [2, RH, K]
                S1 = stm[c, 1, :, :, hb, :]
                S2 = stm[c, 2, :, :, hb, :]
                zeta[b] = S0.sum(axis=(0, 1))
                Sx[b] = S1.sum(axis=(0, 1))
                Sy[b] = (np.float32(RG) * S2 + roff[:, :, None] * S0).sum(axis=(0, 1))
    return zeta, Sx, Sy


def _epilogue(zeta, Sx, Sy):
    cum_x = np.cumsum(Sx.reshape(-1), dtype=np.float32).reshape(B, K)
    cum_y = np.cumsum(Sy.reshape(-1), dtype=np.float32).reshape(B, K)
    kx = np.round(cum_x / zeta)
    ky = np.round(cum_y / zeta)
    kx = np.where((kx > W) | (kx < 0), np.float32(W * 0.5), kx).astype(np.float32)
    ky = np.where((ky > H) | (ky < 0), np.float32(H * 0.5), ky).astype(np.float32)
    return np.stack([kx, ky], axis=-1).astype(np.float32)


def kernel(combined_hm_preds, batch_size=B, num_of_kp=K, **_unused):
    x = np.ascontiguousarray(np.asarray(combined_hm_preds), dtype=np.float32)
    assert x.shape == (B, K, H, W), x.shape
    map_out, zeta, Sx, Sy, _ = run_device(x)
    keypoint = _epilogue(zeta, Sx, Sy)
    return map_out, keypoint, zeta


# revision 43
# speedup vs baseline: 2.6309x; 1.4403x over previous
"""Trainium2 Bass kernel for DetectionConfidenceMap2keypoint.

Computes, for x [B=32, K=64, H=96, W=128] fp32:
  map = softmax(x, axis=1)  (channel softmax per (b, i, j))
  zeta[b,k] = sum_{i,j} map
  Sx[b,k]   = sum_{i,j} map * j
  Sy[b,k]   = sum_{i,j} map * i
  keypoint  = round(cumsum over flattened (b,k) of Sx|Sy / zeta), clamped

Strategy: pure data parallel over batch (4 batches per core, 8 cores).
Per core, batches are processed in pairs stacked on the partition dim
(2 x 64 channels = 128 partitions).  For each image row i the [128(k), 128(j)]
tile is PE-transposed to [128(j), 128(2b x k)], exp'd on ScalarE
(PSUM -> SBUF), the per-(j,b) denominators reduced on VectorE, normalized
with a stride-0-broadcast tensor_tensor multiply, PE-transposed back and
copied to an SBUF staging buffer for DMA out.  zeta/Sx/Sy come from
accumulating PE matmuls (bf16) with weights [1, j, g] against the transposed
normalized map; the tiny flattened-(b,k) cumsum epilogue runs on host.
"""

import sys

import numpy as np

for _p in ("/opt/trn_rl_repo",):
    if _p not in sys.path:
        sys.path.insert(0, _p)

B, K, H, W = 32, 64, 96, 128
NCORES = 8
BPC = B // NCORES  # batches per core
NPAIRS = BPC // 2  # batch pairs per core
RG = 8  # image rows per group (two PSUM banks)
RH = RG // 2  # rows per stats half-accumulator (one PSUM bank)
NG = H // RG  # groups per batch pair
CHUNK_ROWS = 16  # image rows per DMA chunk
GPC = CHUNK_ROWS // RG  # groups per chunk
NCHUNK = H // CHUNK_ROWS
FD = RG * W  # free dim of one group tile (1024)
FDH = RH * W  # free dim of one stats half (512)

# fp32 PE matmuls run at 1/4 rate (f32r needs producer-side rounding, which
# the BIR verifier enforces).  Instead the stats matmul runs in bf16: the
# normalized map is cast fp32->bf16 on the otherwise-idle GPSIMD engine.
# Stats are sums of ~12k bf16-rounded terms accumulated in fp32 PSUM, so the
# relative error is ~4e-3/sqrt(12288) ~ 4e-5 — verified against the
# reference in test.py (keypoints must match exactly).
STATS_BF16 = True


def _build_nc(repeat=1):
    import concourse.bass as bass
    import concourse.tile as tile
    from concourse import bacc, mybir
    from concourse.masks import make_identity

    f32 = mybir.dt.float32

    # Bacc (not raw Bass): its compile() pass splits multi-semaphore waits
    # into EventSemaphore prefix instructions — TRN2 instructions can carry
    # at most one wait, and Tile alone emits more.
    wdt = mybir.dt.bfloat16 if STATS_BF16 else f32
    nc = bacc.Bacc("TRN2", target_bir_lowering=False, debug=False)
    x = nc.dram_tensor("x", [BPC, K, H, W], f32, kind="ExternalInput").ap()
    wst = nc.dram_tensor("wst", [128, NG, 3], wdt, kind="ExternalInput").ap()
    y = nc.dram_tensor("y", [BPC, K, H, W], f32, kind="ExternalOutput").ap()
    st = nc.dram_tensor("st", [NPAIRS, 3, 2, FDH], f32, kind="ExternalOutput").ap()
    zt = nc.dram_tensor("zt", [128, NPAIRS], f32, kind="ExternalOutput").ap()

    with tile.TileContext(nc) as tc:
        for _ in range(repeat):
            _kernel_body(tc, x, wst, y, st, zt, bass, mybir, make_identity)
    nc.compile()
    return nc


def _kernel_body(tc, x, wst, y, st, zt, bass, mybir, make_identity):
    nc = tc.nc
    f32 = mybir.dt.float32
    bf16 = mybir.dt.bfloat16
    Exp = mybir.ActivationFunctionType.Exp
    Copy = mybir.ActivationFunctionType.Copy
    X = mybir.AxisListType.X
    mult = mybir.AluOpType.mult

    import contextlib

    ctx = contextlib.ExitStack()
    with ctx:
        consts = ctx.enter_context(tc.tile_pool(name="consts", bufs=1))
        inpool = ctx.enter_context(tc.tile_pool(name="inpool", bufs=4))
        epool = ctx.enter_context(tc.tile_pool(name="epool", bufs=6))
        mpool = ctx.enter_context(tc.tile_pool(name="mpool", bufs=6))
        dpool = ctx.enter_context(tc.tile_pool(name="dpool", bufs=8))
        bfpool = ctx.enter_context(tc.tile_pool(name="bfpool", bufs=6))
        stagepool = ctx.enter_context(tc.tile_pool(name="stagepool", bufs=4))
        statsb = ctx.enter_context(tc.tile_pool(name="statsb", bufs=1))
        # PSUM bank budget (8 banks): psumf 2x2 + psumb 2x1 + stats 2x1.
        psumf = ctx.enter_context(tc.tile_pool(name="psumf", bufs=2, space="PSUM"))
        psumb = ctx.enter_context(tc.tile_pool(name="psumb", bufs=2, space="PSUM"))
        psums = ctx.enter_context(tc.tile_pool(name="psums", bufs=2, space="PSUM"))

        wdt = bf16 if STATS_BF16 else f32
        ident = consts.tile([128, 128], f32)
        make_identity(nc, ident)
        wst_sb = consts.tile([128, NG, 3], wdt)
        nc.sync.dma_start(out=wst_sb, in_=wst)
        stats_sb = statsb.tile([3, NPAIRS, 2, FDH], f32)
        zeta_sb = statsb.tile([128, NPAIRS], f32)

        pending = None  # phase-B work item carried across one group

        def phase_b(it):
            c = it["c"]
            for h in range(2):
                # stats matmul first: it carries the wait on the stats rhs, so
                # the back transposes below need no extra foreign-sem waits.
                nc.tensor.matmul(
                    it["stats_ps"][h],
                    it["w"],
                    it["stats_rhs"][:, h * RH : (h + 1) * RH, :].rearrange(
                        "p a b -> p (a b)"
                    ),
                    start=(it["g"] == 0),
                    stop=(it["g"] == NG - 1),
                )
                psb = psumb.tile([128, RH, 128], f32, tag="psb")
                for r in range(RH):
                    nc.tensor.transpose(
                        psb[:, r, :], it["mapT"][:, h * RH + r, :], ident
                    )
                # the stage copy doubles as the fp32 zeta accumulator: the
                # per-partition (= per (b,k)) sum of the copied map lands in
                # one column of zparts for free.
                nc.scalar.activation(
                    out=it["stage"][
                        :, it["gg"] * RG + h * RH : it["gg"] * RG + (h + 1) * RH, :
                    ],
                    in_=psb,
                    func=Copy,
                    accum_out=it["zp"][:, 2 * it["g"] + h : 2 * it["g"] + h + 1],
                )
            if it["g"] == NG - 1:
                for h in range(2):
                    nc.scalar.copy(out=stats_sb[:, c, h, :], in_=it["stats_ps"][h])
                nc.sync.dma_start(out=st[c], in_=stats_sb[:, c, :, :])
                nc.vector.reduce_sum(
                    out=zeta_sb[:, c : c + 1], in_=it["zp"], axis=X
                )
                if c == NPAIRS - 1:
                    nc.sync.dma_start(out=zt, in_=zeta_sb)
            if it["gg"] == GPC - 1:
                ch = it["ch"]
                nc.sync.dma_start(
                    out=y[
                        2 * c : 2 * c + 2, :, ch * CHUNK_ROWS : (ch + 1) * CHUNK_ROWS, :
                    ].rearrange("b k r w -> (b k) r w"),
                    in_=it["stage"],
                )

        for c in range(NPAIRS):
            stats_ps = [
                psums.tile([3, FDH], f32, tag="stats", name=f"stats_ps{c}_{h}")
                for h in range(2)
            ]
            zp = dpool.tile([128, 2 * NG], f32, tag="zp")
            for ch in range(NCHUNK):
                in_t = inpool.tile([128, CHUNK_ROWS, W], f32)
                nc.sync.dma_start(
                    out=in_t,
                    in_=x[
                        2 * c : 2 * c + 2, :, ch * CHUNK_ROWS : (ch + 1) * CHUNK_ROWS, :
                    ].rearrange("b k r w -> (b k) r w"),
                )
                stage = stagepool.tile([128, CHUNK_ROWS, W], f32)
                for gg in range(GPC):
                    g = ch * GPC + gg
                    # ---- phase A: fwd transpose, exp, denom, recip, normalize
                    psf = psumf.tile([128, RG, 128], f32)
                    for r in range(RG):
                        nc.tensor.transpose(
                            psf[:, r, :], in_t[:, gg * RG + r, :], ident
                        )
                    eT = epool.tile([128, RG, 128], f32)
                    nc.scalar.activation(out=eT, in_=psf, func=Exp)
                    den = dpool.tile([128, 2 * RG], f32)
                    nc.vector.reduce_sum(
                        out=den,
                        in_=eT.rearrange("p a (s k) -> p (a s) k", k=64),
                        axis=X,
                    )
                    rcp = dpool.tile([128, 2 * RG], f32)
                    nc.vector.reciprocal(out=rcp, in_=den)
                    mapT = mpool.tile([128, RG, 128], f32)
                    nc.vector.tensor_tensor(
                        out=mapT.rearrange("p a b -> p (a b)"),
                        in0=eT.rearrange("p a b -> p (a b)"),
                        in1=rcp.unsqueeze(2).to_broadcast([128, 2 * RG, 64]),
                        op=mult,
                    )
                    if STATS_BF16:
                        stats_rhs = bfpool.tile([128, RG, 128], bf16)
                        nc.gpsimd.tensor_copy(out=stats_rhs, in_=mapT)
                    else:
                        stats_rhs = mapT
                    item = dict(
                        mapT=mapT,
                        stats_rhs=stats_rhs,
                        stage=stage,
                        stats_ps=stats_ps,
                        zp=zp,
                        w=wst_sb[:, g, :],
                        g=g,
                        gg=gg,
                        ch=ch,
                        c=c,
                    )
                    # ---- phase B of the previous group (software pipeline)
                    if pending is not None:
                        phase_b(pending)
                    pending = item
        phase_b(pending)


_CACHE = {}


def _get_nc():
    if "nc" not in _CACHE:
        _CACHE["nc"] = _build_nc()
    return _CACHE["nc"]


def _host_weights():
    w = np.zeros((128, NG, 3), dtype=np.float32)
    w[:, :, 0] = 1.0
    w[:, :, 1] = np.arange(128, dtype=np.float32)[:, None]
    w[:, :, 2] = np.arange(NG, dtype=np.float32)[None, :]
    if STATS_BF16:
        import ml_dtypes

        w = w.astype(ml_dtypes.bfloat16)  # 1, j<=127, g<=11: all exact in bf16
    return w


def run_device(x, trace=False, tmpdir=None):
    """Run the bass kernel on 8 cores. x: [B,K,H,W] fp32.

    Returns (map_out [B,K,H,W], zeta, Sx, Sy  each [B,K]), exec_time_ns."""
    from concourse.bass_utils import run_bass_kernel_spmd

    nc = _get_nc()
    wst = _host_weights()
    in_maps = [
        {"x": np.ascontiguousarray(x[BPC * m : BPC * (m + 1)]), "wst": wst}
        for m in range(NCORES)
    ]
    kw = {}
    if tmpdir is not None:
        kw["tmpdir"] = tmpdir
    res = run_bass_kernel_spmd(
        nc, in_maps, core_ids=list(range(NCORES)), trace=trace, **kw
    )
    map_out = np.concatenate([r["y"] for r in res.results], axis=0)
    zeta, Sx, Sy = decode_stats([r["st"] for r in res.results])
    # overwrite zeta with the exact-fp32 accumulator path
    for m, r in enumerate(res.results):
        ztm = r["zt"].reshape(2, K, NPAIRS)
        for c in range(NPAIRS):
            for hb in range(2):
                zeta[BPC * m + 2 * c + hb] = ztm[hb, :, c]
    return map_out, zeta, Sx, Sy, res.exec_time_ns


def decode_stats(st_list):
    """st per core: [NPAIRS, 3, 2(half), FDH] where the FDH cols are
    (r' in RH, b in 2, k in 64) and the accumulators are summed over groups g
    with weights [1, j, g].  Row index i = RG*g + RH*h + r'."""
    zeta = np.empty((B, K), np.float32)
    Sx = np.empty((B, K), np.float32)
    Sy = np.empty((B, K), np.float32)
    roff = (
        np.float32(RH) * np.arange(2, dtype=np.float32)[:, None]
        + np.arange(RH, dtype=np.float32)[None, :]
    )
    for m, st in enumerate(st_list):
        stm = st.reshape(NPAIRS, 3, 2, RH, 2, K)
        for c in range(NPAIRS):
            for hb in range(2):
                b = BPC * m + 2 * c + hb
                S0 = stm[c, 0, :, :, hb, :]  # [2, RH, K]
                S1 = stm[c, 1, :, :, hb, :]
                S2 = stm[c, 2, :, :, hb, :]
                zeta[b] = S0.sum(axis=(0, 1))
                Sx[b] = S1.sum(axis=(0, 1))
                Sy[b] = (np.float32(RG) * S2 + roff[:, :, None] * S0).sum(axis=(0, 1))
    return zeta, Sx, Sy


def _epilogue(zeta, Sx, Sy):
    cum_x = np.cumsum(Sx.reshape(-1), dtype=np.float32).reshape(B, K)
    cum_y = np.cumsum(Sy.reshape(-1), dtype=np.float32).reshape(B, K)
    kx = np.round(cum_x / zeta)
    ky = np.round(cum_y / zeta)
    kx = np.where((kx > W) | (kx < 0), np.float32(W * 0.5), kx).astype(np.float32)
    ky = np.where((ky > H) | (ky < 0), np.float32(H * 0.5), ky).astype(np.float32)
    return np.stack([kx, ky], axis=-1).astype(np.float32)


def kernel(combined_hm_preds, batch_size=B, num_of_kp=K, **_unused):
    x = np.ascontiguousarray(np.asarray(combined_hm_preds), dtype=np.float32)
    assert x.shape == (B, K, H, W), x.shape
    map_out, zeta, Sx, Sy, _ = run_device(x)
    keypoint = _epilogue(zeta, Sx, Sy)
    return map_out, keypoint, zeta
